# revision 22
# baseline (speedup 1.0000x reference)
"""AffineTransformationsToMatrix Trainium2 kernel (raw Bass, 8-core SPMD).

Input:  vector [B, 6] f32 rows = (tx, ty, tz, ax, ay, az)
Output: [B, 12] f32 rows of [R - I | t] (3x4 row-major), R = Rx @ Ry @ Rz.

Closed form (c* = cos, s* = sin):
  out0  = cy*cz - 1        out1  = -cy*sz           out2  = sy       out3  = tx
  out4  = cx*sz+sx*sy*cz   out5  = cx*cz-sx*sy*sz-1 out6  = -sx*cy   out7  = ty
  out8  = sx*sz-cx*sy*cz   out9  = sx*cz+cx*sy*sz   out10 = cx*cy-1  out11 = tz

The ACT Sin LUT only accepts [-pi, pi], so angles are range-reduced with the
magic-number rounding trick (k = round(a/2pi) via +1.5*2^23), and cosines use
cos(a) = sin(pi/2 - |a'|) with |a'| from one abs_max op.

Per-core layout: batch rows 128-partition-major, TILE_N rows/partition/tile.
Software-pipelined across 4 engines (all waits are standalone wait_ge
instructions; DMA instructions carry only their completion increment):
  SP : input DMAs + output DMAs (HWDGE ring)
  ACT: range-reduce r1/r2, 6 Sin LUT evals, sy, translation copy
  DVE: range-reduce r3 (stt), P/Q product blocks, 2 fused add/sub combine ops
  GPS: abs_max, sy*(cz,sz) products, cy*(cz,-sz) -> out[0,1], out[10,6]
       copies, diagonal -1
Stage order per tile i: in-DMA -> ACT-A(r1,r2) -> DVE-A(r3) -> GPS-A(r4)
  -> ACT-B(sins) -> GPS-B1(v5,v1) -> DVE-B(v3,v4,v6a,v6b) -> GPS-B2(a11,a12)
  -> out-DMA.  Double-buffered; engine k runs stage A of tile i while running
stage B of tile i-1.

M tile segments (xTILE_N): 0=sz 1=cz 2=cy 3=sy*cz 4=sy*sz 5=-sz
P = cx*M[0:5] @ PQ[0:5n] = [cx*sz, cx*cz, cx*cy, cx*sycz, cx*sysz]
Q = sx*M[0:5] @ PQ[5n:10n] = [sx*sz, sx*cz, sx*cy, sx*sycz, sx*sysz]
"""

import numpy as np

import concourse.bass as bass
import concourse.mybir as mybir

B = 4194304
N_CORES = 8
B_LOC = B // N_CORES  # 524288
P = 128
TILE_N = 512

F32 = mybir.dt.float32
PI = float(np.pi)
HALF_PI = float(np.pi / 2)
TWO_PI = float(2 * np.pi)
INV_2PI = float(1.0 / (2 * np.pi))
MAGIC = 12582912.0  # 1.5 * 2^23: adding+subtracting rounds fp32 to nearest int
Sin = mybir.ActivationFunctionType.Sin
Copy = mybir.ActivationFunctionType.Copy
Ident = mybir.ActivationFunctionType.Identity
Abs = mybir.ActivationFunctionType.Abs
Alu = mybir.AluOpType


def _fap(t, off, dims, w):
    """AP on a [128, w] SBUF tensor handle: partition dim + custom free dims.
    off/steps in f32 elements within a partition row."""
    return bass.AP(t, off, [[w, P]] + [list(d) for d in dims])


def build_nc(b_loc=B_LOC, tile_n=TILE_N):
    rpp = b_loc // P
    n = tile_n
    nt = rpp // n
    assert rpp % n == 0 and nt >= 3

    nc = bass.Bass("TRN2")
    for value in (HALF_PI, -HALF_PI, PI, -PI, -1.0, MAGIC, -MAGIC):
        t = nc.alloc_sbuf_tensor(f"const-f32-{value}", [128, 1], F32)
        nc.gpsimd.memset(t.ap(), value)
        nc.const_aps.aps[(F32, value)] = t.ap()
    nc.all_engine_barrier()

    vec = nc.dram_tensor("vector", [b_loc, 6], F32, kind="ExternalInput")
    out = nc.dram_tensor("out", [b_loc, 12], F32, kind="ExternalOutput")
    vec_v = vec[:, :].rearrange("(p r) c -> p (r c)", p=P)   # [128, rpp*6]
    out_v = out[:, :].rearrange("(p r) c -> p (r c)", p=P)   # [128, rpp*12]

    import contextlib
    ctx = contextlib.ExitStack()
    with ctx:
        sb = lambda name, w: [
            ctx.enter_context(nc.sbuf_tensor(f"{name}{b}", [P, w], F32))
            for b in range(2)
        ]
        inb = sb("inb", 6 * n)
        M = sb("m", 6 * n)
        PQ = sb("pq", 10 * n)
        outb = sb("outb", 12 * n)
        KT = sb("kt", 3 * n)   # k values, then reused for |x'| (WT)
        XT = sb("xt", 3 * n)   # range-reduced angles [x', y', z']
        cxt = sb("cx", n)
        sxt = sb("sx", n)

        sem = lambda name: ctx.enter_context(nc.semaphore(name))
        in_dma = [sem("in_dma0"), sem("in_dma1")]
        out_dma = [sem("out_dma0"), sem("out_dma1")]
        act_a = sem("act_a")
        act_r = sem("act_r")
        act_s = sem("act_s")
        dve_r = sem("dve_r")
        dve_e = sem("dve_e")
        gps_v = sem("gps_v")
        gps_e = sem("gps_e")

        # angles (ax, ay, az) strided in the input tile, shape (3, n)
        ang = lambda b: _fap(inb[b], 3, [[1, 3], [6, n]], 6 * n)

        block = ctx.enter_context(nc.Block())

        @block.sync
        def _(sync):
            for i in range(nt):
                if i >= 2:
                    sync.wait_ge(gps_e, 2 * (i - 1))
                    j = i - 2
                    sync.dma_start(
                        out=out_v[:, j * 12 * n:(j + 1) * 12 * n],
                        in_=outb[j % 2][:, :],
                    ).then_inc(out_dma[j % 2], 16)
                sync.dma_start(
                    out=inb[i % 2][:, :],
                    in_=vec_v[:, i * 6 * n:(i + 1) * 6 * n],
                ).then_inc(in_dma[i % 2], 16)
            for j in (nt - 2, nt - 1):
                sync.wait_ge(gps_e, 2 * (j + 1))
                sync.dma_start(
                    out=out_v[:, j * 12 * n:(j + 1) * 12 * n],
                    in_=outb[j % 2][:, :],
                ).then_inc(out_dma[j % 2], 16)
            sync.wait_ge(out_dma[0], 16 * ((nt + 1) // 2))
            sync.wait_ge(out_dma[1], 16 * (nt // 2))

        @block.scalar
        def _(scalar):
            for i in range(nt + 1):
                if i < nt:  # stage A(i): range-reduce k
                    b = i % 2
                    kt = KT[b][:, :]
                    scalar.wait_ge(in_dma[b], 16 * (i // 2 + 1))
                    scalar.activation(kt, ang(b), Ident, bias=MAGIC, scale=INV_2PI).then_inc(act_a, 1)
                    scalar.wait_ge(act_a, 2 * i + 1)
                    scalar.activation(kt, kt, Ident, bias=-MAGIC).then_inc(act_a, 1)
                if i >= 1:  # stage B(j): |x'| then LUT evals
                    j = i - 1
                    b = j % 2
                    mm = lambda k: _fap(M[b], k * n, [[1, n]], 6 * n)
                    xt = lambda c: _fap(XT[b], c * n, [[1, n]], 3 * n)
                    wt = lambda c: _fap(KT[b], c * n, [[1, n]], 3 * n)
                    ob = lambda off, dims: _fap(outb[b], off, dims, 12 * n)

                    scalar.wait_ge(dve_r, j + 1)
                    scalar.activation(KT[b][:, :], XT[b][:, :], Abs).then_inc(act_r, 1)
                    scalar.wait_ge(act_r, j + 1)
                    scalar.activation(mm(0), xt(2), Sin)                      # sz
                    scalar.activation(mm(1), wt(2), Sin, bias=HALF_PI, scale=-1.0)  # cz
                    scalar.activation(mm(2), wt(1), Sin, bias=HALF_PI, scale=-1.0)  # cy
                    scalar.activation(mm(5), xt(2), Sin, scale=-1.0)          # -sz
                    scalar.activation(cxt[b][:, :], wt(0), Sin, bias=HALF_PI, scale=-1.0)
                    scalar.activation(sxt[b][:, :], xt(0), Sin)
                    if j >= 2:
                        scalar.wait_ge(out_dma[b], 16 * (j // 2))
                    scalar.activation(ob(2, [[12, n]]), xt(1), Sin).then_inc(act_s, 1)  # sy

        @block.vector
        def _(vector):
            for i in range(nt + 1):
                if i < nt:  # stage A(i): x' = a - 2*pi*k
                    b = i % 2
                    vector.wait_ge(act_a, 2 * (i + 1))
                    vector.scalar_tensor_tensor(
                        out=_fap(XT[b], 0, [[n, 3], [1, n]], 3 * n),
                        in0=KT[b][:, :].rearrange("p (c n) -> p c n", c=3),
                        scalar=-TWO_PI,
                        in1=ang(b),
                        op0=Alu.mult,
                        op1=Alu.add,
                    ).then_inc(dve_r, 1)
                if i >= 1:  # stage B(j): products and combines
                    j = i - 1
                    b = j % 2
                    mf = lambda off, dims: _fap(M[b], off, dims, 6 * n)
                    pq = lambda off, dims: _fap(PQ[b], off, dims, 10 * n)
                    ob = lambda off, dims: _fap(outb[b], off, dims, 12 * n)

                    vector.wait_ge(gps_v, j + 1)
                    # P = cx * M[0:5]
                    vector.tensor_mul(
                        out=pq(0, [[n, 5], [1, n]]),
                        in0=_fap(cxt[b], 0, [[0, 5], [1, n]], n),
                        in1=mf(0, [[n, 5], [1, n]]),
                    )
                    # Q = sx * M[0:5]
                    vector.tensor_mul(
                        out=pq(5 * n, [[n, 5], [1, n]]),
                        in0=_fap(sxt[b], 0, [[0, 5], [1, n]], n),
                        in1=mf(0, [[n, 5], [1, n]]),
                    ).then_inc(dve_e, 1)
                    vector.wait_ge(dve_e, 2 * j + 1)
                    # out4 = P0 + Q3 ; out9 = Q1 + P4
                    vector.tensor_add(
                        out=ob(4, [[5, 2], [12, n]]),
                        in0=pq(0, [[6 * n, 2], [1, n]]),
                        in1=pq(8 * n, [[-4 * n, 2], [1, n]]),
                    )
                    # out5 = P1 - Q4 ; out8 = Q0 - P3
                    vector.tensor_sub(
                        out=ob(5, [[3, 2], [12, n]]),
                        in0=pq(n, [[4 * n, 2], [1, n]]),
                        in1=pq(9 * n, [[-6 * n, 2], [1, n]]),
                    ).then_inc(dve_e, 1)

        @block.gpsimd
        def _(gpsimd):
            for j in range(nt):
                b = j % 2
                mf = lambda off, dims: _fap(M[b], off, dims, 6 * n)
                pq = lambda off, dims: _fap(PQ[b], off, dims, 10 * n)
                ob = lambda off, dims: _fap(outb[b], off, dims, 12 * n)

                gpsimd.wait_ge(act_s, j + 1)
                if j >= 2:
                    gpsimd.wait_ge(out_dma[b], 16 * (j // 2))
                # out[0,1] = cy * [cz, -sz]
                gpsimd.tensor_mul(
                    out=ob(0, [[1, 2], [12, n]]),
                    in0=mf(2 * n, [[0, 2], [1, n]]),
                    in1=mf(n, [[4 * n, 2], [1, n]]),
                )
                # M[3,4] = sy * [cz, sz]
                gpsimd.tensor_mul(
                    out=mf(3 * n, [[n, 2], [1, n]]),
                    in0=_fap(outb[b], 2, [[0, 2], [12, n]], 12 * n),
                    in1=mf(n, [[-n, 2], [1, n]]),
                ).then_inc(gps_v, 1)
                # translation: out[3,7,11] <- in[0,1,2]
                gpsimd.tensor_copy(
                    out=ob(3, [[4, 3], [12, n]]),
                    in_=_fap(inb[b], 0, [[1, 3], [6, n]], 6 * n),
                )
                gpsimd.wait_ge(dve_e, 2 * (j + 1))
                # out10 = P2 ; out6 = -Q2  (Pool has no tensor_scalar opcode,
                # so scalar operands come from a broadcast const AP)
                neg1 = nc.const_aps.aps[(F32, -1.0)]
                gpsimd.tensor_copy(
                    out=ob(10, [[12, n]]), in_=pq(2 * n, [[1, n]])
                ).then_inc(gps_e, 1)
                gpsimd.tensor_mul(
                    out=ob(6, [[12, n]]),
                    in0=pq(7 * n, [[1, n]]),
                    in1=neg1.to_broadcast((P, n)),
                )
                # diagonal -= 1 (needs out0 from v5 and out10 from the copy
                # above to have fully retired on this engine)
                gpsimd.wait_ge(gps_e, 2 * j + 1)
                diag = ob(0, [[5, 3], [12, n]])
                gpsimd.tensor_add(
                    out=diag, in0=diag, in1=neg1.to_broadcast((P, 3, n))
                ).then_inc(gps_e, 1)

    return nc


_NC_CACHE = {}


def _get_nc(b_loc=B_LOC, tile_n=TILE_N):
    key = (b_loc, tile_n)
    if key not in _NC_CACHE:
        _NC_CACHE[key] = build_nc(b_loc, tile_n)
    return _NC_CACHE[key]


def kernel(vector: np.ndarray) -> np.ndarray:
    from concourse.bass_utils import run_bass_kernel_spmd

    vector = np.ascontiguousarray(np.asarray(vector, dtype=np.float32))
    assert vector.shape == (B, 6), vector.shape
    nc = _get_nc()
    shards = np.split(vector, N_CORES, axis=0)
    in_maps = [{"vector": s} for s in shards]
    res = run_bass_kernel_spmd(nc, in_maps, core_ids=list(range(N_CORES)))
    return np.concatenate([r["out"] for r in res.results], axis=0)


# revision 32
# speedup vs baseline: 1.7668x; 1.7668x over previous
"""AffineTransformationsToMatrix Trainium2 kernel (raw Bass, 8-core SPMD).

Input:  vector [B, 6] f32 rows = (tx, ty, tz, ax, ay, az)
Output: [B, 12] f32 rows of [R - I | t] (3x4 row-major), R = Rx @ Ry @ Rz.

Closed form (c* = cos, s* = sin):
  out0  = cy*cz - 1        out1  = -cy*sz           out2  = sy       out3  = tx
  out4  = cx*sz+sx*sy*cz   out5  = cx*cz-sx*sy*sz-1 out6  = -sx*cy   out7  = ty
  out8  = sx*sz-cx*sy*cz   out9  = sx*cz+cx*sy*sz   out10 = cx*cy-1  out11 = tz

ACT's Sin LUT only accepts [-pi, pi]: angles are range-reduced with
magic-number rounding (k = round(a/2pi) via +1.5*2^23), and cosines use
cos(a) = sin(pi/2 - |a'|).

Per-core: batch rows 128-partition-major, TILE_N rows/partition/tile,
double-buffered, software-pipelined across SP (input DMA), DVE and ACT
(which also issues output DMAs). GpSimd is left idle: any GpSimd SBUF op
takes the DVE/GpSimd shared port lock and stalls DVE 1:1, so work moved
there is zero-sum. Per tile j:

  DVE-A(j): d1  K3 = (angles*INV_2PI)+MAGIC     [ts fused, deinterleave]
            d3  XT = (K3*-2pi)+angles           [stt]
            d4  WT = abs_max(XT,0)              [ts]
  ACT-A(j): 6 Sin LUT evals -> M, sy -> out2, t-copy -> out{3,7,11}
  DVE-B(j): d5  M[sycz,sysz] = sy*[cz,sz]
            d6  P = cx*M[0:5]  d7  Q = sx*M[0:5]
            d8  out{0,1} = cy*[cz,-sz]
            d9  out{4,9} = [P0+Q3, Q1+P4]
            d10 out5 = (P1-1)-Q4 [stt]   d11 out8 = Q0-P3
  ACT-B(j): out10 = P2-1 [bias], out6 = -Q2 [scale], out0 -= 1, out-DMA

M segments (xTILE_N): 0=sz 1=cz 2=cy 3=sycz 4=sysz 5=-sz 6=cx 7=sx
P = cx*M[0:5] @ PQ[0:5n],  Q = sx*M[0:5] @ PQ[5n:10n]

Every DMA carries only its completion increment (HW DMA instr has one wait
slot); all waits are standalone wait_ge. Same-engine RAW pairs get explicit
self-semaphore syncs (engines pipeline; in-order retire assumed for sems).
"""

import numpy as np

import concourse.bass as bass
import concourse.mybir as mybir

B = 4194304
N_CORES = 8
B_LOC = B // N_CORES  # 524288
P = 128
TILE_N = 512

F32 = mybir.dt.float32
PI = float(np.pi)
HALF_PI = float(np.pi / 2)
TWO_PI = float(2 * np.pi)
INV_2PI = float(1.0 / (2 * np.pi))
MAGIC = 12582912.0  # 1.5 * 2^23
Sin = mybir.ActivationFunctionType.Sin
Copy = mybir.ActivationFunctionType.Copy
Ident = mybir.ActivationFunctionType.Identity
Alu = mybir.AluOpType


def _fap(t, off, dims, w):
    return bass.AP(t, off, [[w, P]] + [list(d) for d in dims])


def build_nc(b_loc=B_LOC, tile_n=TILE_N):
    rpp = b_loc // P
    n = tile_n
    nt = rpp // n
    assert rpp % n == 0 and nt >= 3

    nc = bass.Bass("TRN2")
    for value in (HALF_PI, -HALF_PI, PI, -PI, -1.0, MAGIC, -MAGIC):
        t = nc.alloc_sbuf_tensor(f"const-f32-{value}", [128, 1], F32)
        nc.gpsimd.memset(t.ap(), value)
        nc.const_aps.aps[(F32, value)] = t.ap()
    nc.all_engine_barrier()

    vec = nc.dram_tensor("vector", [b_loc, 6], F32, kind="ExternalInput")
    out = nc.dram_tensor("out", [b_loc, 12], F32, kind="ExternalOutput")
    vec_v = vec[:, :].rearrange("(p r) c -> p (r c)", p=P)
    out_v = out[:, :].rearrange("(p r) c -> p (r c)", p=P)

    import contextlib
    ctx = contextlib.ExitStack()
    with ctx:
        sb = lambda name, w: [
            ctx.enter_context(nc.sbuf_tensor(f"{name}{b}", [P, w], F32))
            for b in range(2)
        ]
        inb = sb("inb", 6 * n)
        M = sb("m", 8 * n)
        PQ = sb("pq", 10 * n)
        outb = sb("outb", 12 * n)
        K3 = sb("k3", 3 * n)
        XT = sb("xt", 3 * n)
        WT = sb("wt", 3 * n)

        sem = lambda name: ctx.enter_context(nc.semaphore(name))
        in_dma = [sem("in_dma0"), sem("in_dma1")]
        out_dma = [sem("out_dma0"), sem("out_dma1")]
        dve_a = sem("dve_a")
        dve_s = sem("dve_s")
        dve_e = sem("dve_e")
        act_r = sem("act_r")
        act_s = sem("act_s")
        act_d = sem("act_d")

        ang = lambda b: _fap(inb[b], 3, [[1, 3], [6, n]], 6 * n)
        t7_tick = {}  # j -> dve_s value after d7(j), filled by the DVE builder

        block = ctx.enter_context(nc.Block())

        @block.sync
        def _(sync):
            for i in range(nt):
                if i >= 2:
                    sync.wait_ge(act_d, i - 1)
                sync.dma_start(
                    out=inb[i % 2][:, :],
                    in_=vec_v[:, i * 6 * n:(i + 1) * 6 * n],
                ).then_inc(in_dma[i % 2], 16)
            sync.wait_ge(out_dma[0], 16 * ((nt + 1) // 2))
            sync.wait_ge(out_dma[1], 16 * (nt // 2))

        @block.vector
        def _(vector):
            c = 0  # dve_s build-time counter

            def selfsync(inst):
                nonlocal c
                c += 1
                inst.then_inc(dve_s, 1)
                return c

            for it in range(nt + 1):
                if it < nt:  # stage A(i): range reduction
                    i = it
                    b = i % 2
                    vector.wait_ge(in_dma[b], 16 * (i // 2 + 1))
                    t1 = selfsync(vector.tensor_scalar(
                        out=_fap(K3[b], 0, [[n, 3], [1, n]], 3 * n),
                        in0=ang(b),
                        scalar1=INV_2PI, scalar2=MAGIC,
                        op0=Alu.mult, op1=Alu.add))
                    vector.wait_ge(dve_s, t1)
                    # d2: K3 = (K3 - MAGIC) * -2pi   [= -2pi * round(a/2pi)]
                    t2 = selfsync(vector.tensor_scalar(
                        out=_fap(K3[b], 0, [[1, 3 * n]], 3 * n),
                        in0=_fap(K3[b], 0, [[1, 3 * n]], 3 * n),
                        scalar1=MAGIC, scalar2=-TWO_PI,
                        op0=Alu.subtract, op1=Alu.mult))
                    vector.wait_ge(dve_s, t2)
                    # d3: XT = K3 + angles  (|x'| is taken on ACT)
                    vector.tensor_add(
                        out=_fap(XT[b], 0, [[n, 3], [1, n]], 3 * n),
                        in0=_fap(K3[b], 0, [[n, 3], [1, n]], 3 * n),
                        in1=ang(b),
                    ).then_inc(dve_a, 1)
                if it >= 1:  # stage B(j): products and combines
                    j = it - 1
                    b = j % 2
                    mf = lambda off, dims: _fap(M[b], off, dims, 8 * n)
                    pq = lambda off, dims: _fap(PQ[b], off, dims, 10 * n)
                    ob = lambda off, dims: _fap(outb[b], off, dims, 12 * n)

                    vector.wait_ge(act_s, j + 1)
                    # d5: M[3,4] = sy * [cz, sz]
                    t5 = selfsync(vector.tensor_mul(
                        out=mf(3 * n, [[n, 2], [1, n]]),
                        in0=_fap(outb[b], 2, [[0, 2], [12, n]], 12 * n),
                        in1=mf(n, [[-n, 2], [1, n]])))
                    vector.wait_ge(dve_s, t5)
                    # d6: P = cx * M[0:5]
                    vector.tensor_mul(
                        out=pq(0, [[n, 5], [1, n]]),
                        in0=mf(6 * n, [[0, 5], [1, n]]),
                        in1=mf(0, [[n, 5], [1, n]]))
                    # d7: Q = sx * M[0:5]
                    t7 = selfsync(vector.tensor_mul(
                        out=pq(5 * n, [[n, 5], [1, n]]),
                        in0=mf(7 * n, [[0, 5], [1, n]]),
                        in1=mf(0, [[n, 5], [1, n]])))
                    t7_tick[j] = t7
                    if j >= 2:
                        vector.wait_ge(out_dma[b], 16 * (j // 2))
                    # d8: out[0,1] = cy * [cz, -sz]
                    vector.tensor_mul(
                        out=ob(0, [[1, 2], [12, n]]),
                        in0=mf(2 * n, [[0, 2], [1, n]]),
                        in1=mf(n, [[4 * n, 2], [1, n]]))
                    vector.wait_ge(dve_s, t7)
                    # d9: out[4,9] = [P0+Q3, Q1+P4]
                    vector.tensor_add(
                        out=ob(4, [[5, 2], [12, n]]),
                        in0=pq(0, [[6 * n, 2], [1, n]]),
                        in1=pq(8 * n, [[-4 * n, 2], [1, n]]))
                    # d10: out5 = (P1 - 1) - Q4
                    vector.scalar_tensor_tensor(
                        out=ob(5, [[12, n]]),
                        in0=pq(n, [[1, n]]),
                        scalar=1.0,
                        in1=pq(9 * n, [[1, n]]),
                        op0=Alu.subtract, op1=Alu.subtract)
                    # d11: out8 = Q0 - P3
                    vector.tensor_sub(
                        out=ob(8, [[12, n]]),
                        in0=pq(5 * n, [[1, n]]),
                        in1=pq(3 * n, [[1, n]]),
                    ).then_inc(dve_e, 1)

        @block.scalar
        def _(scalar):
            for it in range(nt + 1):
                if it < nt:  # stage A(j): LUT evals + copies
                    j = it
                    b = j % 2
                    mm = lambda k: _fap(M[b], k * n, [[1, n]], 8 * n)
                    xt = lambda s: _fap(XT[b], s * n, [[1, n]], 3 * n)
                    wt = lambda s: _fap(WT[b], s * n, [[1, n]], 3 * n)
                    ob = lambda off, dims: _fap(outb[b], off, dims, 12 * n)

                    scalar.wait_ge(dve_a, j + 1)
                    # WT = |XT| (cosine args); XT-reading Sins overlap the drain
                    scalar.activation(WT[b][:, :], XT[b][:, :],
                                      mybir.ActivationFunctionType.Abs).then_inc(act_r, 1)
                    scalar.activation(mm(0), xt(2), Sin)                            # sz
                    scalar.activation(mm(5), xt(2), Sin, scale=-1.0)                # -sz
                    scalar.activation(mm(7), xt(0), Sin)                            # sx
                    scalar.wait_ge(act_r, j + 1)
                    scalar.activation(mm(1), wt(2), Sin, bias=HALF_PI, scale=-1.0)  # cz
                    scalar.activation(mm(2), wt(1), Sin, bias=HALF_PI, scale=-1.0)  # cy
                    scalar.activation(mm(6), wt(0), Sin, bias=HALF_PI, scale=-1.0)  # cx
                    if j >= 2:
                        scalar.wait_ge(out_dma[b], 16 * (j // 2))
                    scalar.activation(ob(2, [[12, n]]), xt(1), Sin).then_inc(act_s, 1)  # sy
                    scalar.activation(                                              # t copy
                        ob(3, [[4, 3], [12, n]]),
                        _fap(inb[b], 0, [[1, 3], [6, n]], 6 * n),
                        Copy)
                if it >= 1:  # stage B(j): PQ epilogue + out-DMA
                    j = it - 1
                    b = j % 2
                    pq = lambda off, dims: _fap(PQ[b], off, dims, 10 * n)
                    ob = lambda off, dims: _fap(outb[b], off, dims, 12 * n)

                    scalar.wait_ge(dve_s, t7_tick[j])  # d7(j) retired
                    # out10 = P2 - 1
                    scalar.activation(ob(10, [[12, n]]), pq(2 * n, [[1, n]]), Ident, bias=-1.0)
                    # out6 = -Q2
                    scalar.activation(ob(6, [[12, n]]), pq(7 * n, [[1, n]]), Copy, scale=-1.0)
                    scalar.wait_ge(dve_e, j + 1)  # d8..d11(j) retired
                    # out0 -= 1
                    diag0 = ob(0, [[12, n]])
                    scalar.activation(diag0, diag0, Ident, bias=-1.0).then_inc(act_d, 1)
                    scalar.wait_ge(act_d, j + 1)
                    scalar.dma_start(
                        out=out_v[:, j * 12 * n:(j + 1) * 12 * n],
                        in_=outb[b][:, :],
                    ).then_inc(out_dma[b], 16)

    return nc


_NC_CACHE = {}


def _get_nc(b_loc=B_LOC, tile_n=TILE_N):
    key = (b_loc, tile_n)
    if key not in _NC_CACHE:
        _NC_CACHE[key] = build_nc(b_loc, tile_n)
    return _NC_CACHE[key]


def kernel(vector: np.ndarray) -> np.ndarray:
    from concourse.bass_utils import run_bass_kernel_spmd

    vector = np.ascontiguousarray(np.asarray(vector, dtype=np.float32))
    assert vector.shape == (B, 6), vector.shape
    nc = _get_nc()
    shards = np.split(vector, N_CORES, axis=0)
    in_maps = [{"vector": s} for s in shards]
    res = run_bass_kernel_spmd(nc, in_maps, core_ids=list(range(N_CORES)))
    return np.concatenate([r["out"] for r in res.results], axis=0)
